# revision 14
# baseline (speedup 1.0000x reference)
"""Trainium2 Bass kernel for the CustomGRU problem (redesign v2).

Reference semantics (fp32):
    z = sigmoid(x_t @ Wz_x + bz + h @ Wz_h)
    r = sigmoid(x_t @ Wr_x + br + h @ Wr_h)
    h~ = tanh(x_t @ Wh_x + bh + (r*h) @ Wh_h)
    h  = (1-z)*h + z*h~            (T=512 steps)
    out = h_T @ Wfc + bfc

Sharding: pure data parallel over batch (8192 -> 8 cores x 1024); the
recurrence runs locally per core; tiny weights replicated.

Per-core design (hidden-major, B=1024 split into G=2 groups of Bg=512):

g-space state: g = (1+h)/2 in [0,1], so tanh(a) = 2*sigmoid(2a)-1 turns
ALL THREE gate nonlinearities into sigmoids (one [97]-row sigma covers
r and z), with the affine shifts folded into pre-scaled weights/biases.
Per step per group (state g materialized in Gamma tiles @base 0):

  MM-rz-g : prz[97,Bg] = w_g^T @ Gamma_{t-2}      (start, opens early)
  MM-rz-mx: prz += w_mx^T @ Wwin_t[0:41]          (stop; rows: m_{t-1}@0-32,
            x_t@33-40 -- the "m-fold": 2W^T g_{t-1} = 2W^T g_{t-2}
            + 2W^T m_{t-1}, so the recurrence cycle goes through the
            cheap m product, not the materialized-state add)
  sigma-rz: U[97,Bg] = sigmoid(prz + b_rz)   (r@0-32, junk@33-63, z@64-96)
  q = r * g_{t-1}   (DVE @0; overlapped by the r-correction matmul)
  MM-h~rc : ph[33,Bg] = w_rc^T @ U[0:33]          (start; -2Wh_h r term)
  MM-h~   : ph += w_h^T @ Qwin[0:41]  (q@0-32, x@33-40)  (stop)
  sigma-t : V[0:33] = sigmoid(ph + 2bh)           (t = sig(2 a_h))
  d = t - g_{t-1}   (DVE, ins @0, out V@64)
  m = z * d         (DVE, ins @64, out -> Wwin_{t+1}@0: next rz closer)
  g_t = g_{t-1} + m (DVE, all @0, into a fresh Gamma tile)

Probed HW rules respected throughout: partition bases in {0,32,64} only,
no AP crossing partition 64 except from base 0, tensor_tensor input pairs
share a base, matmul lhsT/rhs share a base, and a PSUM accumulation group
never mixes two different base partitions (mixing crashes the device).
x is staged in 8-step blocks with one DMA per block per tile kind.
"""

import sys

sys.path.insert(0, "/opt/trn_rl_repo")

from contextlib import ExitStack

import ml_dtypes  # noqa: F401  (registers bfloat16 with numpy)
import numpy as np
import orjson

import concourse.bacc as bacc
import concourse.bass as bass
import concourse.tile as tile
from concourse import mybir
from concourse.bass_utils import run_bass_kernel_spmd

N_CORES = 8
I_IN = 8
H = 33
HOR = 24

AF = mybir.ActivationFunctionType
DT = mybir.dt
F16 = np.float16

XBLK = 8  # steps per x staging block


# --------------------------------------------------------------------------
# walrus in this container rejects CTRL (Drain) instructions carrying more
# than one sync wait; Tile's kernel-tail drain always has several. Split
# them at the serialized-JSON level (mutating the live module corrupts it).
def _split_multiwait_drains(raw: bytes, max_waits: int = 1) -> bytes:
    m = orjson.loads(raw)
    changed = False
    for f in m["functions"]:
        for bb in f["blocks"]:
            out = []
            for inst in bb["instructions"]:
                si = inst.get("sync_info")
                ow = (si or {}).get("on_wait") or []
                if inst.get("opcode") == "Drain" and len(ow) > max_waits:
                    head, tail = ow[:-max_waits], ow[-max_waits:]
                    for k, w in enumerate(head):
                        clone = dict(inst)
                        clone["name"] = f"{inst['name']}-sw{k}"
                        clone["sync_info"] = {"on_update": [], "on_wait": [w]}
                        out.append(clone)
                    inst = dict(inst)
                    inst["sync_info"] = {
                        "on_update": si.get("on_update") or [],
                        "on_wait": tail,
                    }
                    changed = True
                out.append(inst)
            bb["instructions"] = out
    return orjson.dumps(m) if changed else raw


def _install_bir_patch(nc):
    orig = nc.to_json_bytes
    nc.to_json_bytes = lambda: _split_multiwait_drains(orig())


# --------------------------------------------------------------------------
def build_gru_nc(B: int, T: int, finalize: bool = True, G: int = 2,
                 repeat: int = 1, stage: int = 9):
    """Build the per-core Bass module (B = per-core batch).
    stage: debug bisection level (9 = full kernel)."""
    nc = bacc.Bacc("TRN2", target_bir_lowering=False, debug=False)
    f32 = DT.float32
    f16 = DT.float16
    Bg = B // G
    assert T % XBLK == 0 and B % G == 0
    NBLK = T // XBLK

    # host x layout: xS[b, i, k, n] = x[n, b*XBLK + k, i]  (n: per-core batch)
    xS = nc.dram_tensor(
        "xS", [NBLK, I_IN, XBLK, B], f16, kind="ExternalInput"
    ).ap()
    w_g = nc.dram_tensor("w_g", [H, 97], f16, kind="ExternalInput").ap()
    w_mx = nc.dram_tensor("w_mx", [41, 97], f16, kind="ExternalInput").ap()
    w_h = nc.dram_tensor("w_h", [41, H], f16, kind="ExternalInput").ap()
    w_rc = nc.dram_tensor("w_rc", [H, H], f16, kind="ExternalInput").ap()
    b_rz = nc.dram_tensor("b_rz", [97, 1], f32, kind="ExternalInput").ap()
    b_t = nc.dram_tensor("b_t", [H, 1], f32, kind="ExternalInput").ap()
    w_fc = nc.dram_tensor("w_fc", [H, HOR], f16, kind="ExternalInput").ap()
    b_fc = nc.dram_tensor("b_fc", [HOR, 1], f32, kind="ExternalInput").ap()
    y = nc.dram_tensor("y", [HOR, B], f32, kind="ExternalOutput").ap()

    with tile.TileContext(nc) as tc:
        with ExitStack() as ctx:
            consts = ctx.enter_context(tc.tile_pool(name="consts", bufs=1))
            wpool = ctx.enter_context(tc.tile_pool(name="wpool", bufs=2))
            qpool = ctx.enter_context(tc.tile_pool(name="qpool", bufs=2))
            upool = ctx.enter_context(tc.tile_pool(name="upool", bufs=3))
            vpool = ctx.enter_context(tc.tile_pool(name="vpool", bufs=3))
            mpool = ctx.enter_context(tc.tile_pool(name="mpool", bufs=3))
            fpool = ctx.enter_context(tc.tile_pool(name="fpool", bufs=4))
            psum_rz = ctx.enter_context(
                tc.tile_pool(name="psum_rz", bufs=2, space="PSUM"))
            psum_h = ctx.enter_context(
                tc.tile_pool(name="psum_h", bufs=2, space="PSUM"))

            # ---- constants ----
            wg_t = consts.tile([H, 97], f16)
            wmx_t = consts.tile([41, 97], f16)
            wh_t = consts.tile([41, H], f16)
            wrc_t = consts.tile([H, H], f16)
            brz_t = consts.tile([97, 1], f32)
            bt_t = consts.tile([H, 1], f32)
            wfc_t = consts.tile([H, HOR], f16)
            bfc_t = consts.tile([HOR, 1], f32)
            for tl, src in [(wg_t, w_g), (wmx_t, w_mx), (wh_t, w_h),
                            (wrc_t, w_rc), (brz_t, b_rz), (bt_t, b_t),
                            (wfc_t, w_fc), (bfc_t, b_fc)]:
                nc.sync.dma_start(tl[:], src[:])

            BW = XBLK * Bg  # block width in columns

            def run_recurrence(rep):
                wblk = [dict() for _ in range(G)]  # blk idx -> tile
                qblk = [dict() for _ in range(G)]

                def ensure_blocks(g, b):
                    if b >= NBLK or b in wblk[g]:
                        return
                    wt = wpool.tile([41, BW], f16, tag=f"W{g}",
                                    name=f"W{g}_{rep}_{b}")
                    qt = qpool.tile([41, BW], f16, tag=f"Q{g}",
                                    name=f"Q{g}_{rep}_{b}")
                    src = xS[b, :, :, g * Bg:(g + 1) * Bg]
                    dst_w = wt[33:41, :].rearrange("p (k n) -> p k n", n=Bg)
                    dst_q = qt[33:41, :].rearrange("p (k n) -> p k n", n=Bg)
                    nc.sync.dma_start(dst_w, src)
                    nc.sync.dma_start(dst_q, src)
                    wblk[g][b] = wt
                    qblk[g][b] = qt

                def wwin(g, t):
                    """[41, Bg] column window of the W block for step t."""
                    b, k = t // XBLK, t % XBLK
                    return wblk[g][b][:, k * Bg:(k + 1) * Bg]

                # state tiles: Gamma_t holds materialized g_t at base 0
                gm2 = [None] * G   # Gamma_{t-2}
                gm1 = [None] * G   # Gamma_{t-1}
                for g in range(G):
                    ensure_blocks(g, 0)
                    ensure_blocks(g, 1)
                    # window 0: m_{-1} = 0
                    nc.vector.memset(wwin(g, 0)[0:H, :], 0.0)
                    t0 = fpool.tile([H, Bg], f16, tag=f"F{g}",
                                    name=f"Ginit2_{g}_{rep}")
                    t1 = fpool.tile([H, Bg], f16, tag=f"F{g}",
                                    name=f"Ginit1_{g}_{rep}")
                    nc.vector.memset(t0[:, :], 0.5)
                    nc.vector.memset(t1[:, :], 0.5)
                    gm2[g] = t0
                    gm1[g] = t1

                for t in range(T):
                    for g in range(G):
                        b, k = t // XBLK, t % XBLK
                        if k == 0:
                            ensure_blocks(g, b + 1)
                        win = slice(k * Bg, (k + 1) * Bg)
                        # --- rz matmuls: g-part opens early (Gamma_{t-2}),
                        # m/x-part closes (m_{t-1} is the cycle-critical
                        # input, in window t rows 0-32; x at rows 33-40) ---
                        prz = psum_rz.tile([97, Bg], f32, tag=f"prz{g}",
                                           name=f"prz{g}_{rep}_{t}")
                        nc.tensor.matmul(prz[:, :], wg_t[0:H, :],
                                         gm2[g][0:H, :],
                                         start=True, stop=False)
                        nc.tensor.matmul(prz[:, :], wmx_t[0:41, :],
                                         wwin(g, t)[0:41, :],
                                         start=False, stop=True)
                        u = upool.tile([97, Bg], f16, tag=f"U{g}",
                                       name=f"U{g}_{rep}_{t}")
                        nc.scalar.activation(u[0:97, :], prz[0:97, :],
                                             AF.Sigmoid, bias=brz_t[:])
                        gprev = gm1[g][0:H, :]   # materialized g_{t-1} @0
                        if stage < 2:
                            gm2[g], gm1[g] = gm1[g], gm2[g]
                            continue
                        # --- q = r * g_{t-1} -> Q window @0 ---
                        qt_full = qblk[g][t // XBLK]
                        nc.vector.tensor_mul(qt_full[0:H, win], u[0:H, :],
                                             gprev)
                        # --- h~ matmuls: r-correction first (overlaps the
                        # q DVE op), then the q-dependent main part closes ---
                        ph = psum_h.tile([H, Bg], f32, tag=f"ph{g}",
                                         name=f"ph{g}_{rep}_{t}")
                        nc.tensor.matmul(ph[:, :], wrc_t[:, :], u[0:H, :],
                                         start=True, stop=False)
                        nc.tensor.matmul(ph[:, :], wh_t[:, :],
                                         qt_full[0:41, win],
                                         start=False, stop=True)
                        v = vpool.tile([97, Bg], f16, tag=f"V{g}",
                                       name=f"V{g}_{rep}_{t}")
                        nc.scalar.activation(v[0:H, :], ph[:, :], AF.Sigmoid,
                                             bias=bt_t[:])
                        if stage < 3:
                            gm2[g], gm1[g] = gm1[g], gm2[g]
                            continue
                        # --- d = t - g_{t-1}  (ins @0, out @64) ---
                        nc.vector.tensor_sub(v[64:97, :], v[0:H, :], gprev)
                        # --- m = z * d (ins @64) -> next window rows 0-32:
                        # feeds the next step's rz closer directly ---
                        if t + 1 < T:
                            mdst = wwin(g, t + 1)[0:H, :]
                        else:
                            mt = mpool.tile([H, Bg], f16, tag=f"M{g}",
                                            name=f"M{g}_{rep}_{t}")
                            mdst = mt[0:H, :]
                        nc.vector.tensor_mul(mdst, u[64:97, :], v[64:97, :])
                        if stage < 5:
                            gm2[g], gm1[g] = gm1[g], gm2[g]
                            continue
                        # --- g_t = g_{t-1} + m  (all @0) ---
                        gnew = fpool.tile([H, Bg], f16, tag=f"F{g}",
                                          name=f"G{g}_{rep}_{t}")
                        nc.vector.tensor_add(gnew[0:H, :], gprev, mdst)
                        gm2[g] = gm1[g]
                        gm1[g] = gnew
                return gm1

            for rep in range(repeat):
                g_final = run_recurrence(rep)

            # ---- final FC: out = g_T @ (2 Wfc) + (bfc - colsum(Wfc)) ----
            for g in range(G):
                pfc = psum_h.tile([HOR, Bg], f32, tag=f"ph{g}", name=f"pfc{g}")
                nc.tensor.matmul(pfc[:, :], wfc_t[:, :], g_final[g][0:H, :],
                                 start=True, stop=True)
                y_sb = upool.tile([HOR, Bg], f32, tag=f"U{g}", name=f"ysb{g}")
                nc.scalar.activation(y_sb[0:HOR, :], pfc[:, :], AF.Identity,
                                     bias=bfc_t[:])
                nc.sync.dma_start(y[:, g * Bg:(g + 1) * Bg], y_sb[0:HOR, :])

    if finalize:
        nc.finalize()
        _install_bir_patch(nc)
    return nc


# --------------------------------------------------------------------------
def prep_weights(Wz, bz, Wr, br, Wh, bh, Wfc, bfc):
    """Pre-scaled g-space weights (see module docstring)."""
    Wr, Wz, Wh = (np.asarray(a, np.float64) for a in (Wr, Wz, Wh))
    br, bz, bh = (np.asarray(a, np.float64) for a in (br, bz, bh))
    Wfc = np.asarray(Wfc, np.float64)
    bfc = np.asarray(bfc, np.float64)
    Wr_x, Wr_h = Wr[:I_IN], Wr[I_IN:]
    Wz_x, Wz_h = Wz[:I_IN], Wz[I_IN:]
    Wh_x, Wh_h = Wh[:I_IN], Wh[I_IN:]

    w_g = np.zeros((H, 97), np.float64)
    w_g[:, 0:H] = 2.0 * Wr_h
    w_g[:, 64:97] = 2.0 * Wz_h
    w_mx = np.zeros((41, 97), np.float64)
    w_mx[0:H, 0:H] = 2.0 * Wr_h
    w_mx[0:H, 64:97] = 2.0 * Wz_h
    w_mx[33:41, 0:H] = Wr_x
    w_mx[33:41, 64:97] = Wz_x

    b_rz = np.zeros((97, 1), np.float64)
    b_rz[0:H, 0] = br - Wr_h.sum(0)
    b_rz[64:97, 0] = bz - Wz_h.sum(0)

    w_h = np.zeros((41, H), np.float64)
    w_h[0:H] = 4.0 * Wh_h
    w_h[33:41] = 2.0 * Wh_x
    w_rc = -2.0 * Wh_h
    b_t = 2.0 * bh

    w_fc = 2.0 * Wfc
    b_fc = bfc - Wfc.sum(0)
    return {
        "w_g": w_g.astype(F16),
        "w_mx": w_mx.astype(F16),
        "w_h": w_h.astype(F16),
        "w_rc": np.ascontiguousarray(w_rc).astype(F16),
        "b_rz": b_rz.astype(np.float32),
        "b_t": b_t.reshape(H, 1).astype(np.float32),
        "w_fc": np.ascontiguousarray(w_fc).astype(F16),
        "b_fc": b_fc.reshape(HOR, 1).astype(np.float32),
    }


def prepare_in_maps(x, Wz, bz, Wr, br, Wh, bh, Wfc, bfc, n_cores=N_CORES):
    B_total, T, _ = x.shape
    B = B_total // n_cores
    wmap = prep_weights(Wz, bz, Wr, br, Wh, bh, Wfc, bfc)
    in_maps = []
    for c in range(n_cores):
        xc = np.asarray(x[c * B:(c + 1) * B])  # [B, T, I]
        # xS[b, i, k, n] = x[n, b*XBLK + k, i]
        xSc = np.ascontiguousarray(
            xc.reshape(B, T // XBLK, XBLK, I_IN).transpose(1, 3, 2, 0)
        ).astype(F16)
        in_maps.append({"xS": xSc, **wmap})
    return in_maps


def run_gru(x, Wz, bz, Wr, br, Wh, bh, Wfc, bfc, n_cores=N_CORES, G=2,
            **spmd_kwargs):
    B_total, T, _ = x.shape
    B = B_total // n_cores
    nc = build_gru_nc(B, T, G=G)
    in_maps = prepare_in_maps(x, Wz, bz, Wr, br, Wh, bh, Wfc, bfc,
                              n_cores=n_cores)
    res = run_bass_kernel_spmd(
        nc, in_maps, core_ids=list(range(n_cores)), **spmd_kwargs
    )
    y = np.concatenate(
        [res.results[c]["y"].T for c in range(n_cores)], axis=0
    ).astype(np.float32)
    return y, res


def kernel(x, Wz, bz, Wr, br, Wh, bh, Wfc, bfc):
    y, _ = run_gru(x, Wz, bz, Wr, br, Wh, bh, Wfc, bfc)
    return y


# revision 15
# speedup vs baseline: 1.1672x; 1.1672x over previous
"""Trainium2 Bass kernel for the CustomGRU problem (redesign v2).

Reference semantics (fp32):
    z = sigmoid(x_t @ Wz_x + bz + h @ Wz_h)
    r = sigmoid(x_t @ Wr_x + br + h @ Wr_h)
    h~ = tanh(x_t @ Wh_x + bh + (r*h) @ Wh_h)
    h  = (1-z)*h + z*h~            (T=512 steps)
    out = h_T @ Wfc + bfc

Sharding: pure data parallel over batch (8192 -> 8 cores x 1024); the
recurrence runs locally per core; tiny weights replicated.

Per-core design (hidden-major, B=1024 split into G=2 groups of Bg=512):

g-space state: g = (1+h)/2 in [0,1], so tanh(a) = 2*sigmoid(2a)-1 turns
ALL THREE gate nonlinearities into sigmoids (one [97]-row sigma covers
r and z), with the affine shifts folded into pre-scaled weights/biases.
Per step per group (state g materialized in Gamma tiles @base 0):

  MM-rz-g : prz[97,Bg] = w_g^T @ Gamma_{t-2}      (start, opens early)
  MM-rz-mx: prz += w_mx^T @ Wwin_t[0:41]          (stop; rows: m_{t-1}@0-32,
            x_t@33-40 -- the "m-fold": 2W^T g_{t-1} = 2W^T g_{t-2}
            + 2W^T m_{t-1}, so the recurrence cycle goes through the
            cheap m product, not the materialized-state add)
  sigma-rz: U[97,Bg] = sigmoid(prz + b_rz)   (r@0-32, junk@33-63, z@64-96)
  q = r * g_{t-1}   (DVE @0; overlapped by the r-correction matmul)
  MM-h~rc : ph[33,Bg] = w_rc^T @ U[0:33]          (start; -2Wh_h r term)
  MM-h~   : ph += w_h^T @ Qwin[0:41]  (q@0-32, x@33-40)  (stop)
  sigma-t : V[0:33] = sigmoid(ph + 2bh)           (t = sig(2 a_h))
  d = t - g_{t-1}   (DVE, ins @0, out V@64)
  m = z * d         (DVE, ins @64, out -> Wwin_{t+1}@0: next rz closer)
  g_t = g_{t-1} + m (DVE, all @0, into a fresh Gamma tile)

Probed HW rules respected throughout: partition bases in {0,32,64} only,
no AP crossing partition 64 except from base 0, tensor_tensor input pairs
share a base, matmul lhsT/rhs share a base, and a PSUM accumulation group
never mixes two different base partitions (mixing crashes the device).
x is staged in 8-step blocks with one DMA per block per tile kind.
"""

import sys

sys.path.insert(0, "/opt/trn_rl_repo")

from contextlib import ExitStack

import ml_dtypes  # noqa: F401  (registers bfloat16 with numpy)
import numpy as np
import orjson

import concourse.bacc as bacc
import concourse.bass as bass
import concourse.tile as tile
from concourse import mybir
from concourse.bass_utils import run_bass_kernel_spmd

N_CORES = 8
I_IN = 8
H = 33
HOR = 24

AF = mybir.ActivationFunctionType
DT = mybir.dt
F16 = np.float16

XBLK = 8  # steps per x staging block


# --------------------------------------------------------------------------
# walrus in this container rejects CTRL (Drain) instructions carrying more
# than one sync wait; Tile's kernel-tail drain always has several. Split
# them at the serialized-JSON level (mutating the live module corrupts it).
def _split_multiwait_drains(raw: bytes, max_waits: int = 1) -> bytes:
    m = orjson.loads(raw)
    changed = False
    for f in m["functions"]:
        for bb in f["blocks"]:
            out = []
            for inst in bb["instructions"]:
                si = inst.get("sync_info")
                ow = (si or {}).get("on_wait") or []
                if inst.get("opcode") == "Drain" and len(ow) > max_waits:
                    head, tail = ow[:-max_waits], ow[-max_waits:]
                    for k, w in enumerate(head):
                        clone = dict(inst)
                        clone["name"] = f"{inst['name']}-sw{k}"
                        clone["sync_info"] = {"on_update": [], "on_wait": [w]}
                        out.append(clone)
                    inst = dict(inst)
                    inst["sync_info"] = {
                        "on_update": si.get("on_update") or [],
                        "on_wait": tail,
                    }
                    changed = True
                out.append(inst)
            bb["instructions"] = out
    return orjson.dumps(m) if changed else raw


def _install_bir_patch(nc):
    orig = nc.to_json_bytes
    nc.to_json_bytes = lambda: _split_multiwait_drains(orig())


# --------------------------------------------------------------------------
def build_gru_nc(B: int, T: int, finalize: bool = True, G: int = 2,
                 repeat: int = 1, stage: int = 9):
    """Build the per-core Bass module (B = per-core batch).
    stage: debug bisection level (9 = full kernel)."""
    nc = bacc.Bacc("TRN2", target_bir_lowering=False, debug=False)
    f32 = DT.float32
    f16 = DT.float16
    Bg = B // G
    assert T % XBLK == 0 and B % G == 0
    NBLK = T // XBLK

    # host x layout: xS[b, i, k, n] = x[n, b*XBLK + k, i]  (n: per-core batch)
    xS = nc.dram_tensor(
        "xS", [NBLK, I_IN, XBLK, B], f16, kind="ExternalInput"
    ).ap()
    w_g = nc.dram_tensor("w_g", [H, 97], f16, kind="ExternalInput").ap()
    w_mx = nc.dram_tensor("w_mx", [41, 97], f16, kind="ExternalInput").ap()
    w_h = nc.dram_tensor("w_h", [41, H], f16, kind="ExternalInput").ap()
    w_rc = nc.dram_tensor("w_rc", [H, H], f16, kind="ExternalInput").ap()
    b_rz = nc.dram_tensor("b_rz", [97, 1], f32, kind="ExternalInput").ap()
    b_t = nc.dram_tensor("b_t", [H, 1], f32, kind="ExternalInput").ap()
    w_fc = nc.dram_tensor("w_fc", [H, HOR], f16, kind="ExternalInput").ap()
    b_fc = nc.dram_tensor("b_fc", [HOR, 1], f32, kind="ExternalInput").ap()
    y = nc.dram_tensor("y", [HOR, B], f32, kind="ExternalOutput").ap()

    with tile.TileContext(nc) as tc:
        with ExitStack() as ctx:
            consts = ctx.enter_context(tc.tile_pool(name="consts", bufs=1))
            wpool = ctx.enter_context(tc.tile_pool(name="wpool", bufs=2))
            qpool = ctx.enter_context(tc.tile_pool(name="qpool", bufs=2))
            upool = ctx.enter_context(tc.tile_pool(name="upool", bufs=3))
            vpool = ctx.enter_context(tc.tile_pool(name="vpool", bufs=3))
            mpool = ctx.enter_context(tc.tile_pool(name="mpool", bufs=3))
            fpool = ctx.enter_context(tc.tile_pool(name="fpool", bufs=4))
            psum_rz = ctx.enter_context(
                tc.tile_pool(name="psum_rz", bufs=2, space="PSUM"))
            psum_h = ctx.enter_context(
                tc.tile_pool(name="psum_h", bufs=2, space="PSUM"))

            # ---- constants ----
            wg_t = consts.tile([H, 97], f16)
            wmx_t = consts.tile([41, 97], f16)
            wh_t = consts.tile([41, H], f16)
            wrc_t = consts.tile([H, H], f16)
            brz_t = consts.tile([97, 1], f32)
            bt_t = consts.tile([H, 1], f32)
            wfc_t = consts.tile([H, HOR], f16)
            bfc_t = consts.tile([HOR, 1], f32)
            for tl, src in [(wg_t, w_g), (wmx_t, w_mx), (wh_t, w_h),
                            (wrc_t, w_rc), (brz_t, b_rz), (bt_t, b_t),
                            (wfc_t, w_fc), (bfc_t, b_fc)]:
                nc.sync.dma_start(tl[:], src[:])

            BW = XBLK * Bg  # block width in columns

            def run_recurrence(rep):
                wblk = [dict() for _ in range(G)]  # blk idx -> tile
                qblk = [dict() for _ in range(G)]

                def ensure_blocks(g, b):
                    if b >= NBLK or b in wblk[g]:
                        return
                    wt = wpool.tile([41, BW], f16, tag=f"W{g}",
                                    name=f"W{g}_{rep}_{b}")
                    qt = qpool.tile([41, BW], f16, tag=f"Q{g}",
                                    name=f"Q{g}_{rep}_{b}")
                    src = xS[b, :, :, g * Bg:(g + 1) * Bg]
                    dst_w = wt[33:41, :].rearrange("p (k n) -> p k n", n=Bg)
                    dst_q = qt[33:41, :].rearrange("p (k n) -> p k n", n=Bg)
                    nc.sync.dma_start(dst_w, src)
                    nc.sync.dma_start(dst_q, src)
                    wblk[g][b] = wt
                    qblk[g][b] = qt

                def wwin(g, t):
                    """[41, Bg] column window of the W block for step t."""
                    b, k = t // XBLK, t % XBLK
                    return wblk[g][b][:, k * Bg:(k + 1) * Bg]

                # state tiles: Gamma_t holds materialized g_t at base 0
                gm2 = [None] * G   # Gamma_{t-2}
                gm1 = [None] * G   # Gamma_{t-1}
                for g in range(G):
                    ensure_blocks(g, 0)
                    ensure_blocks(g, 1)
                    # window 0: m_{-1} = 0
                    nc.vector.memset(wwin(g, 0)[0:H, :], 0.0)
                    t0 = fpool.tile([H, Bg], f16, tag=f"F{g}",
                                    name=f"Ginit2_{g}_{rep}")
                    t1 = fpool.tile([H, Bg], f16, tag=f"F{g}",
                                    name=f"Ginit1_{g}_{rep}")
                    nc.vector.memset(t0[:, :], 0.5)
                    nc.vector.memset(t1[:, :], 0.5)
                    gm2[g] = t0
                    gm1[g] = t1

                u_cur = [None] * G   # sigma-rz output of the current step
                v_cur = [None] * G

                def half_a(g, t):
                    """rz matmuls + sigma-rz + q."""
                    b, k = t // XBLK, t % XBLK
                    if k == 0:
                        ensure_blocks(g, b + 1)
                    win = slice(k * Bg, (k + 1) * Bg)
                    # g-part opens early (Gamma_{t-2}); m/x-part closes
                    # (m_{t-1}, the cycle-critical input, sits in window t
                    # rows 0-32; x at rows 33-40)
                    prz = psum_rz.tile([97, Bg], f32, tag=f"prz{g}",
                                       name=f"prz{g}_{rep}_{t}")
                    nc.tensor.matmul(prz[:, :], wg_t[0:H, :],
                                     gm2[g][0:H, :], start=True, stop=False)
                    nc.tensor.matmul(prz[:, :], wmx_t[0:41, :],
                                     wwin(g, t)[0:41, :],
                                     start=False, stop=True)
                    u = upool.tile([97, Bg], f16, tag=f"U{g}",
                                   name=f"U{g}_{rep}_{t}")
                    nc.scalar.activation(u[0:97, :], prz[0:97, :],
                                         AF.Sigmoid, bias=brz_t[:])
                    u_cur[g] = u
                    gprev = gm1[g][0:H, :]   # materialized g_{t-1} @0
                    # q = r * g_{t-1} -> Q window @0
                    nc.vector.tensor_mul(qblk[g][b][0:H, win], u[0:H, :],
                                         gprev)

                def half_b(g, t):
                    """h~ matmuls + sigma-t + d, m, state add."""
                    b, k = t // XBLK, t % XBLK
                    win = slice(k * Bg, (k + 1) * Bg)
                    u = u_cur[g]
                    gprev = gm1[g][0:H, :]
                    # r-correction first (overlaps the q DVE op), then the
                    # q-dependent main part closes
                    ph = psum_h.tile([H, Bg], f32, tag=f"ph{g}",
                                     name=f"ph{g}_{rep}_{t}")
                    nc.tensor.matmul(ph[:, :], wrc_t[:, :], u[0:H, :],
                                     start=True, stop=False)
                    nc.tensor.matmul(ph[:, :], wh_t[:, :],
                                     qblk[g][b][0:41, win],
                                     start=False, stop=True)
                    v = vpool.tile([97, Bg], f16, tag=f"V{g}",
                                   name=f"V{g}_{rep}_{t}")
                    nc.scalar.activation(v[0:H, :], ph[:, :], AF.Sigmoid,
                                         bias=bt_t[:])
                    # d = t - g_{t-1}  (ins @0, out @64)
                    nc.vector.tensor_sub(v[64:97, :], v[0:H, :], gprev)
                    # m = z * d (ins @64) -> next window rows 0-32: feeds
                    # the next step's rz closer directly
                    if t + 1 < T:
                        mdst = wwin(g, t + 1)[0:H, :]
                    else:
                        mt = mpool.tile([H, Bg], f16, tag=f"M{g}",
                                        name=f"M{g}_{rep}_{t}")
                        mdst = mt[0:H, :]
                    nc.vector.tensor_mul(mdst, u[64:97, :], v[64:97, :])
                    # g_t = g_{t-1} + m  (all @0)
                    gnew = fpool.tile([H, Bg], f16, tag=f"F{g}",
                                      name=f"G{g}_{rep}_{t}")
                    nc.vector.tensor_add(gnew[0:H, :], gprev, mdst)
                    gm2[g] = gm1[g]
                    gm1[g] = gnew

                # software pipeline: group 1 runs half a step behind group 0
                # so the in-order engine queues interleave the two halves.
                for t in range(T):
                    half_a(0, t)
                    if t > 0:
                        half_b(1, t - 1)
                    half_a(1, t)
                    half_b(0, t)
                half_b(1, T - 1)
                return gm1

            for rep in range(repeat):
                g_final = run_recurrence(rep)

            # ---- final FC: out = g_T @ (2 Wfc) + (bfc - colsum(Wfc)) ----
            for g in range(G):
                pfc = psum_h.tile([HOR, Bg], f32, tag=f"ph{g}", name=f"pfc{g}")
                nc.tensor.matmul(pfc[:, :], wfc_t[:, :], g_final[g][0:H, :],
                                 start=True, stop=True)
                y_sb = upool.tile([HOR, Bg], f32, tag=f"U{g}", name=f"ysb{g}")
                nc.scalar.activation(y_sb[0:HOR, :], pfc[:, :], AF.Identity,
                                     bias=bfc_t[:])
                nc.sync.dma_start(y[:, g * Bg:(g + 1) * Bg], y_sb[0:HOR, :])

    if finalize:
        nc.finalize()
        _install_bir_patch(nc)
    return nc


# --------------------------------------------------------------------------
def prep_weights(Wz, bz, Wr, br, Wh, bh, Wfc, bfc):
    """Pre-scaled g-space weights (see module docstring)."""
    Wr, Wz, Wh = (np.asarray(a, np.float64) for a in (Wr, Wz, Wh))
    br, bz, bh = (np.asarray(a, np.float64) for a in (br, bz, bh))
    Wfc = np.asarray(Wfc, np.float64)
    bfc = np.asarray(bfc, np.float64)
    Wr_x, Wr_h = Wr[:I_IN], Wr[I_IN:]
    Wz_x, Wz_h = Wz[:I_IN], Wz[I_IN:]
    Wh_x, Wh_h = Wh[:I_IN], Wh[I_IN:]

    w_g = np.zeros((H, 97), np.float64)
    w_g[:, 0:H] = 2.0 * Wr_h
    w_g[:, 64:97] = 2.0 * Wz_h
    w_mx = np.zeros((41, 97), np.float64)
    w_mx[0:H, 0:H] = 2.0 * Wr_h
    w_mx[0:H, 64:97] = 2.0 * Wz_h
    w_mx[33:41, 0:H] = Wr_x
    w_mx[33:41, 64:97] = Wz_x

    b_rz = np.zeros((97, 1), np.float64)
    b_rz[0:H, 0] = br - Wr_h.sum(0)
    b_rz[64:97, 0] = bz - Wz_h.sum(0)

    w_h = np.zeros((41, H), np.float64)
    w_h[0:H] = 4.0 * Wh_h
    w_h[33:41] = 2.0 * Wh_x
    w_rc = -2.0 * Wh_h
    b_t = 2.0 * bh

    w_fc = 2.0 * Wfc
    b_fc = bfc - Wfc.sum(0)
    return {
        "w_g": w_g.astype(F16),
        "w_mx": w_mx.astype(F16),
        "w_h": w_h.astype(F16),
        "w_rc": np.ascontiguousarray(w_rc).astype(F16),
        "b_rz": b_rz.astype(np.float32),
        "b_t": b_t.reshape(H, 1).astype(np.float32),
        "w_fc": np.ascontiguousarray(w_fc).astype(F16),
        "b_fc": b_fc.reshape(HOR, 1).astype(np.float32),
    }


def prepare_in_maps(x, Wz, bz, Wr, br, Wh, bh, Wfc, bfc, n_cores=N_CORES):
    B_total, T, _ = x.shape
    B = B_total // n_cores
    wmap = prep_weights(Wz, bz, Wr, br, Wh, bh, Wfc, bfc)
    in_maps = []
    for c in range(n_cores):
        xc = np.asarray(x[c * B:(c + 1) * B])  # [B, T, I]
        # xS[b, i, k, n] = x[n, b*XBLK + k, i]
        xSc = np.ascontiguousarray(
            xc.reshape(B, T // XBLK, XBLK, I_IN).transpose(1, 3, 2, 0)
        ).astype(F16)
        in_maps.append({"xS": xSc, **wmap})
    return in_maps


def run_gru(x, Wz, bz, Wr, br, Wh, bh, Wfc, bfc, n_cores=N_CORES, G=2,
            **spmd_kwargs):
    B_total, T, _ = x.shape
    B = B_total // n_cores
    nc = build_gru_nc(B, T, G=G)
    in_maps = prepare_in_maps(x, Wz, bz, Wr, br, Wh, bh, Wfc, bfc,
                              n_cores=n_cores)
    res = run_bass_kernel_spmd(
        nc, in_maps, core_ids=list(range(n_cores)), **spmd_kwargs
    )
    y = np.concatenate(
        [res.results[c]["y"].T for c in range(n_cores)], axis=0
    ).astype(np.float32)
    return y, res


def kernel(x, Wz, bz, Wr, br, Wh, bh, Wfc, bfc):
    y, _ = run_gru(x, Wz, bz, Wr, br, Wh, bh, Wfc, bfc)
    return y
